# revision 1
# baseline (speedup 1.0000x reference)
"""AttentiveMatch kernel for Trainium2 (8 NeuronCores, data-parallel over batch).

Reference math (per batch):
    pn = l2norm(p); qn = l2norm(q)
    w  = -(pn @ qn^T) / D          # [S,S]
    mv = (w @ q) / S               # [S,D]
    mn = l2norm(mv)
    out = -mean(pn * mn, -1)       # [S]

Device pipeline (scalars folded, sign flips cancel):
    G^T  = q @ p^T                       [S,S]   matmul 1 (PSUM, fp32)
    A^T  = diag(1/|q_j|) G^T             scale fused into PSUM->SBUF copy
    M^T  = q^T A                         [D,S]   matmul 2 (lhsT = q natural)
    dot_i = p_i . M_i = sum_j (1/|q_j|) (G^T)^2[j,i]   (matmul with rq weights)
    ss_i  = |M_i|^2  = sum_d (M^T)^2[d,i]              (matmul with ones)
    out_i = (1/D) dot_i / (|p_i| sqrt(ss_i))

Each core handles 8 batches; inputs shipped as bf16 in natural and
transposed layouts; all accumulation fp32.
"""

import os
import sys

for _p in ("/opt/trn_rl_repo",):
    if _p not in sys.path:
        sys.path.append(_p)

import numpy as np
import ml_dtypes

import concourse.bacc as bacc
import concourse.mybir as mybir
import concourse.tile as tile
from concourse.bass_utils import run_bass_kernel_spmd

B, S, D = 64, 512, 768
NCORES = 8
BP = B // NCORES          # batches per core
ST = S // 128             # s tiles (4)
KT = D // 128             # d tiles (6)
F32 = mybir.dt.float32
F32R = mybir.dt.float32r
BF16 = mybir.dt.bfloat16
AF = mybir.ActivationFunctionType
ALU = mybir.AluOpType

_NC = None

if os.environ.get("KERNEL_LDW_OPT", "0") == "1":
    import concourse.bass_utils as _bu

    _orig_run_command = _bu.run_command

    def _patched_run_command(cmd, **kw):
        cmd = [
            ("--enable-ldw-opt=true" if c == "--enable-ldw-opt=false" else c)
            for c in cmd
        ]
        return _orig_run_command(cmd, **kw)

    _bu.run_command = _patched_run_command


def _build():
    nc = bacc.Bacc("TRN2", target_bir_lowering=False, debug=False, num_devices=NCORES)
    pn_d = nc.dram_tensor("pn", [BP, 128, ST * D], BF16, kind="ExternalInput")
    qn_d = nc.dram_tensor("qn", [BP, 128, ST * D], BF16, kind="ExternalInput")
    pt_d = nc.dram_tensor("pt", [BP, 128, KT * S], BF16, kind="ExternalInput")
    qt_d = nc.dram_tensor("qt", [BP, 128, KT * S], BF16, kind="ExternalInput")
    out_d = nc.dram_tensor("out", [128, BP * ST], F32, kind="ExternalOutput")

    with tile.TileContext(nc) as tc:
        with (
            tc.tile_pool(name="cst", bufs=1) as cst,
            tc.tile_pool(name="inp", bufs=3) as inp,
            tc.tile_pool(name="ats", bufs=2) as ats,
            tc.tile_pool(name="gps", bufs=3, space="PSUM") as gps,
            tc.tile_pool(name="mps", bufs=3, space="PSUM") as mps,
            tc.tile_pool(name="rps", bufs=1, space="PSUM") as rps,
            tc.tile_pool(name="tps", bufs=1, space="PSUM") as tps,
            tc.tile_pool(name="scr", bufs=2) as scr,
            tc.tile_pool(name="st", bufs=2) as st,
            tc.tile_pool(name="res", bufs=1) as res,
        ):
            wd = res.tile([128, BP * ST], F32)
            ones16 = cst.tile([128, 1], BF16)
            nc.gpsimd.memset(ones16[:], 1.0)
            onef = cst.tile([128, 1], F32)
            nc.gpsimd.memset(onef[:], 1.0)

            for b in range(BP):
                # qt via sync ring, pt via scalar ring (parallel HWDGE rings);
                # batch 0 split into chunks so mm1 starts on the first arrivals
                qt_c = []
                pt_c = []
                nch = 3 if b == 0 else 1
                w = (KT // nch) * S
                for c in range(nch):
                    qc = inp.tile([128, w], BF16, tag=f"qt{c}_{nch}")
                    nc.sync.dma_start(qc[:], qt_d[b, :, c * w:(c + 1) * w])
                    pc = inp.tile([128, w], BF16, tag=f"pt{c}_{nch}")
                    if b == 0:
                        nc.scalar.dma_start(pc[:], pt_d[b, :, c * w:(c + 1) * w])
                    else:
                        nc.sync.dma_start(pc[:], pt_d[b, :, c * w:(c + 1) * w])
                    qt_c.append(qc)
                    pt_c.append(pc)
                q_t = inp.tile([128, ST * D], BF16, tag="q")
                nc.gpsimd.dma_start(q_t[:], qn_d[b])
                p_t = inp.tile([128, ST * D], BF16, tag="p")
                nc.gpsimd.dma_start(p_t[:], pn_d[b])
                kw = 2 if b == 0 else KT

                # q row sum-of-squares via ACT Square+accumulate (needed for rq)
                ssq_q = st.tile([128, ST], F32, tag="ssq_q")
                for t in range(ST):
                    sl = slice(t * D, (t + 1) * D)
                    aq = scr.tile([128, D], BF16, tag="aq")
                    nc.scalar.activation(aq[:], q_t[:, sl], AF.Square,
                                         accum_out=ssq_q[:, t:t + 1])
                sq_q = st.tile([128, ST], F32, tag="sq_q")
                nc.scalar.activation(sq_q[:], ssq_q[:], AF.Sqrt)
                rq = st.tile([128, ST], F32, tag="rq")
                nc.vector.reciprocal(rq[:], sq_q[:])
                sqq16 = st.tile([128, ST], BF16, tag="sqq16")
                nc.vector.tensor_copy(sqq16[:], sq_q[:])

                rows = rps.tile([64, 512], F32, tag="rows")
                trn = tps.tile([128, 2 * ST], F32, tag="trn")

                # mm1: G^T[j,i] = sum_d q[j,d] p[i,d]; A^T = rq * G^T;
                # dot_i = sum_j sq_q[j] (A^T)^2[j,i]  (== sum_j rq_j G^2)
                at_tiles = []
                h_tiles = []
                for j in range(ST):
                    g = gps.tile([128, S], F32, tag="g")
                    for k in range(KT):
                        kc, ko = divmod(k, kw)
                        nc.tensor.matmul(
                            g[:],
                            lhsT=qt_c[kc][:, ko * S + j * 128: ko * S + (j + 1) * 128],
                            rhs=pt_c[kc][:, ko * S: (ko + 1) * S],
                            start=(k == 0), stop=(k == KT - 1),
                        )
                    at = ats.tile([128, S], BF16, tag=f"at{j}")
                    nc.scalar.activation(at[:], g[:], AF.Copy, scale=rq[:, j:j + 1])
                    at_tiles.append(at)
                    h = scr.tile([128, S], BF16, tag=f"h{j}")
                    nc.vector.tensor_mul(h[:], at[:], at[:])
                    h_tiles.append(h)
                for j in range(ST):
                    nc.tensor.matmul(
                        rows[0:1, :], lhsT=sqq16[:, j:j + 1], rhs=h_tiles[j][:],
                        start=(j == 0), stop=(j == ST - 1),
                    )

                # mm2: M^T[d,i] = sum_j q[j,d] A^T[j,i]; ss_row += ones^T @ (M^T)^2
                # ACT squares PSUM directly; DVE sums pairs -> 3 ones-matmuls
                s2_pair = []
                for k in range(KT):
                    mt = mps.tile([128, S], F32, tag="mt")
                    for jt in range(ST):
                        nc.tensor.matmul(
                            mt[:],
                            lhsT=q_t[:, jt * D + k * 128: jt * D + (k + 1) * 128],
                            rhs=at_tiles[jt][:],
                            start=(jt == 0), stop=(jt == ST - 1),
                        )
                    ms = scr.tile([128, S], BF16, tag="ms")
                    nc.vector.tensor_copy(ms[:], mt[:])
                    s2 = scr.tile([128, S], BF16, tag=f"s2{k % 2}")
                    nc.vector.tensor_mul(s2[:], ms[:], ms[:])
                    s2_pair.append(s2)
                    if k % 2 == 1:
                        s2s = scr.tile([128, S], BF16, tag="s2s")
                        nc.vector.tensor_add(s2s[:], s2_pair[0][:], s2_pair[1][:])
                        s2_pair = []
                        nc.tensor.matmul(
                            rows[32:33, :], lhsT=ones16[:], rhs=s2s[:],
                            start=(k == 1), stop=(k == KT - 1),
                        )

                # p row sum-of-squares (only needed for the finals -> late)
                ssq_p = st.tile([128, ST], F32, tag="ssq_p")
                for t in range(ST):
                    sl = slice(t * D, (t + 1) * D)
                    ap_ = scr.tile([128, D], BF16, tag="ap")
                    nc.scalar.activation(ap_[:], p_t[:, sl], AF.Square,
                                         accum_out=ssq_p[:, t:t + 1])
                sq_p = st.tile([128, ST], F32, tag="sq_p")
                nc.scalar.activation(sq_p[:], ssq_p[:], AF.Sqrt)
                rp = st.tile([128, ST], F32, tag="rp")
                nc.vector.reciprocal(rp[:], sq_p[:])

                # transpose the two [1,512] rows into [128, ST] columns
                rowsb = st.tile([64, 512], F32, tag="rowsb")
                nc.vector.tensor_copy(rowsb[:], rows[:])
                for c in range(ST):
                    nc.tensor.matmul(
                        trn[:, c:c + 1],
                        lhsT=rowsb[0:1, c * 128:(c + 1) * 128],
                        rhs=onef[0:1, :], start=(c == 0), stop=False,
                    )
                for c in range(ST):
                    nc.tensor.matmul(
                        trn[:, ST + c: ST + c + 1],
                        lhsT=rowsb[32:33, c * 128:(c + 1) * 128],
                        rhs=onef[32:33, :], start=(c == 0), stop=(c == ST - 1),
                    )

                # wd = (1/D) * dot / (sq_p * sqrt(ss));  sqrt(D^2 ss) folds 1/D
                sd = st.tile([128, ST], F32, tag="sd")
                nc.scalar.activation(sd[:], trn[:, ST: 2 * ST], AF.Sqrt,
                                     scale=float(D) * float(D))
                rs = st.tile([128, ST], F32, tag="rs")
                nc.vector.reciprocal(rs[:], sd[:])
                w1 = st.tile([128, ST], F32, tag="w1")
                nc.vector.tensor_mul(w1[:], trn[:, 0:ST], rp[:])
                nc.vector.tensor_mul(wd[:, b * ST: (b + 1) * ST], w1[:], rs[:])

            nc.sync.dma_start(out_d[:], wd[:])
    nc.compile()
    return nc


def _get_nc():
    global _NC
    if _NC is None:
        _NC = _build()
    return _NC


def _prep_inputs(p, q):
    p = np.asarray(p, dtype=np.float32)
    q = np.asarray(q, dtype=np.float32)
    p16 = p.astype(ml_dtypes.bfloat16)
    q16 = q.astype(ml_dtypes.bfloat16)

    # natural: [core, b, part, t*D + d] with s = t*128 + part
    def nat(x):
        return np.ascontiguousarray(
            x.reshape(NCORES, BP, ST, 128, D).transpose(0, 1, 3, 2, 4)
        ).reshape(NCORES, BP, 128, ST * D)

    # transposed: [core, b, part, k*S + i] with d = k*128 + part
    def tr(x):
        return np.ascontiguousarray(
            x.reshape(NCORES, BP, S, KT, 128).transpose(0, 1, 4, 3, 2)
        ).reshape(NCORES, BP, 128, KT * S)

    pn, qn, pt, qt = nat(p16), nat(q16), tr(p16), tr(q16)
    return [
        {"pn": pn[c], "qn": qn[c], "pt": pt[c], "qt": qt[c]}
        for c in range(NCORES)
    ]


def _postprocess(results):
    o = np.stack([np.asarray(r["out"], dtype=np.float32) for r in results])
    # o[c, part, b*ST + t] is out for batch c*BP+b at i = t*128 + part
    o = o.reshape(NCORES, 128, BP, ST).transpose(0, 2, 3, 1).reshape(B, 1, S)
    return np.ascontiguousarray(o)


def _run(inputs, trace=False, **kw):
    nc = _get_nc()
    in_maps = _prep_inputs(inputs["p"], inputs["q"])
    res = run_bass_kernel_spmd(nc, in_maps, list(range(NCORES)), trace=trace, **kw)
    return _postprocess(res.results), res


def kernel(p, q):
    out, _ = _run({"p": p, "q": q})
    return out



# revision 12
# speedup vs baseline: 1.6078x; 1.6078x over previous
"""AttentiveMatch kernel for Trainium2 (8 NeuronCores, data-parallel over batch).

Reference math (per batch):
    pn = l2norm(p); qn = l2norm(q)
    w  = -(pn @ qn^T) / D          # [S,S]
    mv = (w @ q) / S               # [S,D]
    mn = l2norm(mv)
    out = -mean(pn * mn, -1)       # [S]

Folded device pipeline (scalars folded, sign flips cancel):
    qs   = sqrt(1/|q_j|) * q_j                      (host)
    G'   = qs @ p^T                 [S,S]  fp8 DoubleRow matmul (G'[j,i])
    g8   = fp8(G')                  PSUM->SBUF copy
    dot_i = sum_j g8[j,i]^2         square (ACT) + adds + ones-matmul
    M    = sum_j g8[j,i] qs[j,d]    [i,d]  fp8 DoubleRow matmul
    ss_i = sum_d M[i,d]^2           square-accumulate along free dim
    out_i = (1/(D |p_i|)) dot_i / sqrt(ss_i)

Each core handles 8 batches; inputs shipped as fp8(e4m3) in transposed
(d-major) and natural (j-major) layouts; all accumulation fp32.
"""

import os
import sys

for _p in ("/opt/trn_rl_repo",):
    if _p not in sys.path:
        sys.path.append(_p)

import numpy as np
import ml_dtypes

import concourse.bacc as bacc
import concourse.mybir as mybir
import concourse.tile as tile
from concourse.bass_utils import run_bass_kernel_spmd

B, S, D = 64, 512, 768
NCORES = 8
BP = B // NCORES          # batches per core
ST = S // 128             # s tiles (4)
KT = D // 128             # d tiles (6)
DC = 2                    # d chunks for mm2 output (2 x 384)
DW = D // DC              # 384
F32 = mybir.dt.float32
BF16 = mybir.dt.bfloat16
F8 = mybir.dt.float8e4
AF = mybir.ActivationFunctionType
ALU = mybir.AluOpType
DR = mybir.MatmulPerfMode.DoubleRow

_NC = None


def _build(ncores=NCORES, do_compile=True):
    nc = bacc.Bacc("TRN2", target_bir_lowering=False, debug=False, num_devices=ncores)
    # transposed layouts: [b, part, k, s] with d = k*128 + part
    pt_d = nc.dram_tensor("pt8", [BP, 128, KT, S], F8, kind="ExternalInput")
    qt_d = nc.dram_tensor("qt8", [BP, 128, KT, S], F8, kind="ExternalInput")
    # natural layout: [b, part, js, d] with j = js*128 + part
    qh_d = nc.dram_tensor("qh8", [BP, 128, ST, D], F8, kind="ExternalInput")
    # 1/(D*|p_i|) at [part, b*ST + t], i = t*128 + part
    rpc_d = nc.dram_tensor("rpc", [128, BP * ST], F32, kind="ExternalInput")
    out_d = nc.dram_tensor("out", [128, BP * ST], F32, kind="ExternalOutput")

    with tile.TileContext(nc) as tc:
        with (
            tc.tile_pool(name="cst", bufs=1) as cst,
            tc.tile_pool(name="inp", bufs=3) as inp,
            tc.tile_pool(name="g8p", bufs=2) as g8p,
            tc.tile_pool(name="hpp", bufs=2) as hpp,
            tc.tile_pool(name="scr", bufs=2) as scr,
            tc.tile_pool(name="res", bufs=1) as res,
            tc.tile_pool(name="gps", bufs=3, space="PSUM") as gps,
            tc.tile_pool(name="mps", bufs=2, space="PSUM") as mps,
            tc.tile_pool(name="dps", bufs=1, space="PSUM") as dps,
        ):
            ones16 = cst.tile([128, 1], BF16)
            nc.gpsimd.memset(ones16[:], 1.0)
            rpc = cst.tile([128, BP * ST], F32)
            nc.sync.dma_start(rpc[:], rpc_d[:])

            # per-i dot products, transposed: col b*ST + t, i = t*128 + part
            dotT = dps.tile([128, 512], F32)
            # ss accumulator columns [i-part, dc*BP*ST + (b,ib)] (dc-major)
            ssc = res.tile([128, DC * BP * ST], F32)

            for b in range(BP):
                q_t = inp.tile([128, KT, S], F8, tag="qt")
                nc.sync.dma_start(q_t[:], qt_d[b])
                p_t = inp.tile([128, KT, S], F8, tag="pt")
                nc.scalar.dma_start(p_t[:], pt_d[b])
                qh_t = inp.tile([128, ST, D], F8, tag="qh")
                nc.gpsimd.dma_start(qh_t[:], qh_d[b])

                g8 = g8p.tile([128, ST, S], F8, tag="g8")
                hp = hpp.tile([128, ST, S], BF16, tag="hp")

                # mm1: G'[j,i] = sum_d qs[j,d] p[i,d]  (fp8 DoubleRow, K=256/mm)
                for jt in range(ST):
                    g = gps.tile([128, S], F32, tag="g")
                    for ks in range(0, KT, 2):
                        nc.tensor.matmul(
                            g[:],
                            lhsT=q_t[:, ks:ks + 2, jt * 128:(jt + 1) * 128],
                            rhs=p_t[:, ks:ks + 2, :],
                            start=(ks == 0), stop=(ks == KT - 2),
                            perf_mode=DR,
                        )
                    # PSUM -> SBUF fp8 copy (mm2 lhsT); then h' = g8^2 (bf16)
                    nc.vector.tensor_copy(g8[:, jt, :], g[:])
                    nc.vector.tensor_mul(hp[:, jt, :], g8[:, jt, :], g8[:, jt, :])

                # mm2: M[i,d] = sum_j g8[j,i] qs[j,d]  (fp8 DoubleRow)
                # output i on partitions so ss reduces along free dim
                for ib in range(ST):
                    mc0 = mps.tile([128, 512], F32, tag="mc0")
                    mc1 = mps.tile([128, 512], F32, tag="mc1")
                    mc = [mc0, mc1]
                    for js in range(0, ST, 2):
                        for dc in range(DC):
                            nc.tensor.matmul(
                                mc[dc][:, 0:DW],
                                lhsT=g8[:, js:js + 2, ib * 128:(ib + 1) * 128],
                                rhs=qh_t[:, js:js + 2, dc * DW:(dc + 1) * DW],
                                start=(js == 0), stop=(js == ST - 2),
                                perf_mode=DR,
                            )
                    col = b * ST + ib
                    # ss chunks: ACT square + accumulate along free dim
                    for dc in range(DC):
                        s2 = scr.tile([128, DW], BF16, tag=f"s2{dc}")
                        acol = dc * BP * ST + col
                        nc.scalar.activation(
                            s2[:], mc[dc][:, 0:DW], AF.Square,
                            accum_out=ssc[:, acol:acol + 1],
                        )

                # dot: sum h' over j-tiles (GPSIMD adds), then per-t ones-matmul
                ha = scr.tile([128, 2, S], BF16, tag="ha")
                nc.gpsimd.tensor_tensor(
                    ha[:], hp[:, 0:2, :], hp[:, 2:4, :], ALU.add
                )
                hs = scr.tile([128, S], BF16, tag="hs")
                nc.gpsimd.tensor_tensor(
                    hs[:], ha[:, 0:1, :], ha[:, 1:2, :], ALU.add
                )
                for t in range(ST):
                    col = b * ST + t
                    nc.tensor.matmul(
                        dotT[:, col:col + 1],
                        lhsT=hs[:, t * 128:(t + 1) * 128],
                        rhs=ones16[:],
                        start=(b == 0 and t == 0),
                        stop=(b == BP - 1 and t == ST - 1),
                    )

            # finals: out = dot * rpc / sqrt(ss0 + ss1)
            nbs = BP * ST
            ssum = res.tile([128, nbs], F32)
            nc.vector.tensor_tensor(ssum[:], ssc[:, 0:nbs], ssc[:, nbs:2 * nbs], ALU.add)
            sd = res.tile([128, nbs], F32)
            nc.scalar.activation(sd[:], ssum[:], AF.Sqrt)
            rs = res.tile([128, nbs], F32)
            nc.vector.reciprocal(rs[:], sd[:])
            w1 = res.tile([128, nbs], F32)
            nc.vector.tensor_tensor(w1[:], dotT[:, 0:nbs], rs[:], ALU.mult)
            wd = res.tile([128, nbs], F32)
            nc.vector.tensor_tensor(wd[:], w1[:], rpc[:], ALU.mult)
            nc.sync.dma_start(out_d[:], wd[:])
    if do_compile:
        nc.compile()
    return nc


def _get_nc():
    global _NC
    if _NC is None:
        _NC = _build()
    return _NC


F8NP = ml_dtypes.float8_e4m3


def _prep_inputs(p, q):
    p = np.asarray(p, dtype=np.float32)
    q = np.asarray(q, dtype=np.float32)

    nq = np.sqrt((q * q).sum(-1))                 # [B,S]
    srq = (1.0 / np.sqrt(nq))[..., None]          # [B,S,1]
    qs = (q * srq).astype(np.float32)
    npn = np.sqrt((p * p).sum(-1))                # [B,S]
    rpc = (1.0 / (float(D) * npn)).astype(np.float32)

    # transposed: [core, b, part, k, s] with d = k*128 + part
    def tr(x):
        x8 = x.astype(F8NP)
        return np.ascontiguousarray(
            x8.reshape(NCORES, BP, S, KT, 128).transpose(0, 1, 4, 3, 2)
        )

    # natural: [core, b, part, js, d] with j = js*128 + part
    def nat(x):
        x8 = x.astype(F8NP)
        return np.ascontiguousarray(
            x8.reshape(NCORES, BP, ST, 128, D).transpose(0, 1, 3, 2, 4)
        )

    pt8, qt8, qh8 = tr(p), tr(qs), nat(qs)
    # rpc: [core, part, b*ST + t], i = t*128 + part
    rpc_l = np.ascontiguousarray(
        rpc.reshape(NCORES, BP, ST, 128).transpose(0, 3, 1, 2)
    ).reshape(NCORES, 128, BP * ST)
    return [
        {"pt8": pt8[c], "qt8": qt8[c], "qh8": qh8[c], "rpc": rpc_l[c]}
        for c in range(NCORES)
    ]


def _postprocess(results):
    o = np.stack([np.asarray(r["out"], dtype=np.float32) for r in results])
    # o[c, part, b*ST + t] is out for batch c*BP+b at i = t*128 + part
    o = o.reshape(NCORES, 128, BP, ST).transpose(0, 2, 3, 1).reshape(B, 1, S)
    return np.ascontiguousarray(o)


def _run(inputs, trace=False, **kw):
    nc = _get_nc()
    in_maps = _prep_inputs(inputs["p"], inputs["q"])
    res = run_bass_kernel_spmd(nc, in_maps, list(range(NCORES)), trace=trace, **kw)
    return _postprocess(res.results), res


def kernel(p, q):
    out, _ = _run({"p": p, "q": q})
    return out


# revision 17
# speedup vs baseline: 1.7731x; 1.1028x over previous
"""AttentiveMatch kernel for Trainium2 (8 NeuronCores, data-parallel over batch).

Reference math (per batch):
    pn = l2norm(p); qn = l2norm(q)
    w  = -(pn @ qn^T) / D          # [S,S]
    mv = (w @ q) / S               # [S,D]
    mn = l2norm(mv)
    out = -mean(pn * mn, -1)       # [S]

Folded device pipeline (scalars folded, sign flips cancel):
    qs   = sqrt(1/|q_j|) * q_j                      (host)
    G'   = qs @ p^T                 [S,S]  fp8 DoubleRow matmul (G'[j,i])
    g8   = fp8(G')                  PSUM->SBUF copy
    dot_i = sum_j g8[j,i]^2         square (ACT) + adds + ones-matmul
    M    = sum_j g8[j,i] qs[j,d]    [i,d]  fp8 DoubleRow matmul
    ss_i = sum_d M[i,d]^2           square-accumulate along free dim
    out_i = (1/(D |p_i|)) dot_i / sqrt(ss_i)

Each core handles 8 batches; inputs shipped as fp8(e4m3) in transposed
(d-major) and natural (j-major) layouts; all accumulation fp32.
"""

import os
import sys

for _p in ("/opt/trn_rl_repo",):
    if _p not in sys.path:
        sys.path.append(_p)

import numpy as np
import ml_dtypes

import concourse.bacc as bacc
import concourse.mybir as mybir
import concourse.tile as tile
from concourse.bass_utils import run_bass_kernel_spmd

B, S, D = 64, 512, 768
NCORES = 8
BP = B // NCORES          # batches per core
ST = S // 128             # s tiles (4)
KT = D // 128             # d tiles (6)
DC = 2                    # d chunks for mm2 output (2 x 384)
DW = D // DC              # 384
F32 = mybir.dt.float32
BF16 = mybir.dt.bfloat16
F8 = mybir.dt.float8e4
AF = mybir.ActivationFunctionType
ALU = mybir.AluOpType
DR = mybir.MatmulPerfMode.DoubleRow

_NC = None


def _build(ncores=NCORES, do_compile=True):
    nc = bacc.Bacc("TRN2", target_bir_lowering=False, debug=False, num_devices=ncores)
    # transposed layouts: [b, part, k, s] with d = k*128 + part
    pt_d = nc.dram_tensor("pt8", [BP, 128, KT, S], F8, kind="ExternalInput")
    qt_d = nc.dram_tensor("qt8", [BP, 128, KT, S], F8, kind="ExternalInput")
    # natural layout: [b, part, js, d] with j = js*128 + part
    qh_d = nc.dram_tensor("qh8", [BP, 128, ST, D], F8, kind="ExternalInput")
    # 1/(D*|p_i|) at [part, b*ST + t], i = t*128 + part
    rpc_d = nc.dram_tensor("rpc", [128, BP * ST], F32, kind="ExternalInput")
    out_d = nc.dram_tensor("out", [128, BP * ST], F32, kind="ExternalOutput")

    with tile.TileContext(nc) as tc:
        with (
            tc.tile_pool(name="cst", bufs=1) as cst,
            tc.tile_pool(name="inp", bufs=3) as inp,
            tc.tile_pool(name="g8p", bufs=2) as g8p,
            tc.tile_pool(name="hpp", bufs=2) as hpp,
            tc.tile_pool(name="scr", bufs=2) as scr,
            tc.tile_pool(name="res", bufs=1) as res,
            tc.tile_pool(name="gps", bufs=3, space="PSUM") as gps,
            tc.tile_pool(name="mps", bufs=2, space="PSUM") as mps,
            tc.tile_pool(name="dps", bufs=1, space="PSUM") as dps,
        ):
            ones16 = cst.tile([128, 1], BF16)
            nc.gpsimd.memset(ones16[:], 1.0)
            rpc = cst.tile([128, BP * ST], F32)
            nc.sync.dma_start(rpc[:], rpc_d[:])

            # per-i dot products, transposed: col b*ST + t, i = t*128 + part
            dotT = dps.tile([128, 512], F32)
            # ss accumulator columns [i-part, b*ST + ib]
            ssc = res.tile([128, BP * ST], F32)

            for b in range(BP):
                q_t = inp.tile([128, KT, S], F8, tag="qt")
                nc.sync.dma_start(q_t[:], qt_d[b])
                p_t = inp.tile([128, KT, S], F8, tag="pt")
                nc.scalar.dma_start(p_t[:], pt_d[b])
                qh_t = inp.tile([128, ST, D], F8, tag="qh")
                nc.gpsimd.dma_start(qh_t[:], qh_d[b])

                g8 = g8p.tile([128, ST, S], F8, tag="g8")
                hp = hpp.tile([128, ST, S], BF16, tag="hp")

                # mm1: G'[j,i] = sum_d qs[j,d] p[i,d]  (fp8 DoubleRow, K=256/mm)
                for jt in range(ST):
                    g = gps.tile([128, S], F32, tag="g")
                    for ks in range(0, KT, 2):
                        nc.tensor.matmul(
                            g[:],
                            lhsT=q_t[:, ks:ks + 2, jt * 128:(jt + 1) * 128],
                            rhs=p_t[:, ks:ks + 2, :],
                            start=(ks == 0), stop=(ks == KT - 2),
                            perf_mode=DR,
                        )
                    # PSUM -> SBUF fp8 copy (mm2 lhsT); then h' ~= G^2 (bf16)
                    nc.vector.tensor_copy(g8[:, jt, :], g[:])
                    if jt < 2:
                        nc.scalar.activation(hp[:, jt, :], g[:], AF.Square)
                    else:
                        nc.vector.scalar_tensor_tensor(
                            hp[:, jt, :], g[:], 1.0, g8[:, jt, :],
                            op0=ALU.mult, op1=ALU.mult,
                        )

                # mm2: M[i,d] = sum_j g8[j,i] qs[j,d]  (fp8 DoubleRow)
                # output i on partitions so ss reduces along free dim
                for ib in range(ST):
                    mc = mps.tile([128, DC, 512], F32, tag="mc")
                    for js in range(0, ST, 2):
                        for dc in range(DC):
                            nc.tensor.matmul(
                                mc[:, dc, 0:DW],
                                lhsT=g8[:, js:js + 2, ib * 128:(ib + 1) * 128],
                                rhs=qh_t[:, js:js + 2, dc * DW:(dc + 1) * DW],
                                start=(js == 0), stop=(js == ST - 2),
                                perf_mode=DR,
                            )
                    col = b * ST + ib
                    # ss: one ACT square + accumulate over both chunks (3D AP)
                    s2 = scr.tile([128, DC * DW], BF16, tag="s2")
                    nc.scalar.activation(
                        s2[:], mc[:, :, 0:DW], AF.Square,
                        accum_out=ssc[:, col:col + 1],
                    )

                # dot: sum h' over j-tiles (DVE bf16 2x adds), then ones-matmuls
                ha = scr.tile([128, 2, S], BF16, tag="ha")
                nc.vector.tensor_tensor(
                    ha[:], hp[:, 0:2, :], hp[:, 2:4, :], ALU.add
                )
                hs = scr.tile([128, S], BF16, tag="hs")
                nc.vector.tensor_tensor(
                    hs[:], ha[:, 0:1, :], ha[:, 1:2, :], ALU.add
                )
                for t in range(ST):
                    col = b * ST + t
                    nc.tensor.matmul(
                        dotT[:, col:col + 1],
                        lhsT=hs[:, t * 128:(t + 1) * 128],
                        rhs=ones16[:],
                        start=(b == 0 and t == 0),
                        stop=(b == BP - 1 and t == ST - 1),
                    )

            # finals: out = dot * rpc / sqrt(ss)
            nbs = BP * ST
            sd = res.tile([128, nbs], F32)
            nc.scalar.activation(sd[:], ssc[:], AF.Sqrt)
            rs = res.tile([128, nbs], F32)
            nc.vector.reciprocal(rs[:], sd[:])
            w1 = res.tile([128, nbs], F32)
            nc.vector.tensor_tensor(w1[:], dotT[:, 0:nbs], rs[:], ALU.mult)
            wd = res.tile([128, nbs], F32)
            nc.vector.tensor_tensor(wd[:], w1[:], rpc[:], ALU.mult)
            nc.sync.dma_start(out_d[:], wd[:])
    if do_compile:
        nc.compile()
    return nc


def _get_nc():
    global _NC
    if _NC is None:
        _NC = _build()
    return _NC


F8NP = ml_dtypes.float8_e4m3


def _prep_inputs(p, q):
    p = np.asarray(p, dtype=np.float32)
    q = np.asarray(q, dtype=np.float32)

    nq = np.sqrt((q * q).sum(-1))                 # [B,S]
    srq = (1.0 / np.sqrt(nq))[..., None]          # [B,S,1]
    qs = (q * srq).astype(np.float32)
    npn = np.sqrt((p * p).sum(-1))                # [B,S]
    rpc = (1.0 / (float(D) * npn)).astype(np.float32)

    # transposed: [core, b, part, k, s] with d = k*128 + part
    def tr(x):
        x8 = x.astype(F8NP)
        return np.ascontiguousarray(
            x8.reshape(NCORES, BP, S, KT, 128).transpose(0, 1, 4, 3, 2)
        )

    # natural: [core, b, part, js, d] with j = js*128 + part
    def nat(x):
        x8 = x.astype(F8NP)
        return np.ascontiguousarray(
            x8.reshape(NCORES, BP, ST, 128, D).transpose(0, 1, 3, 2, 4)
        )

    pt8, qt8, qh8 = tr(p), tr(qs), nat(qs)
    # rpc: [core, part, b*ST + t], i = t*128 + part
    rpc_l = np.ascontiguousarray(
        rpc.reshape(NCORES, BP, ST, 128).transpose(0, 3, 1, 2)
    ).reshape(NCORES, 128, BP * ST)
    return [
        {"pt8": pt8[c], "qt8": qt8[c], "qh8": qh8[c], "rpc": rpc_l[c]}
        for c in range(NCORES)
    ]


def _postprocess(results):
    o = np.stack([np.asarray(r["out"], dtype=np.float32) for r in results])
    # o[c, part, b*ST + t] is out for batch c*BP+b at i = t*128 + part
    o = o.reshape(NCORES, 128, BP, ST).transpose(0, 2, 3, 1).reshape(B, 1, S)
    return np.ascontiguousarray(o)


def _run(inputs, trace=False, **kw):
    nc = _get_nc()
    in_maps = _prep_inputs(inputs["p"], inputs["q"])
    res = run_bass_kernel_spmd(nc, in_maps, list(range(NCORES)), trace=trace, **kw)
    return _postprocess(res.results), res


def kernel(p, q):
    out, _ = _run({"p": p, "q": q})
    return out


# revision 20
# speedup vs baseline: 1.7853x; 1.0068x over previous
"""AttentiveMatch kernel for Trainium2 (8 NeuronCores, data-parallel over batch).

Reference math (per batch):
    pn = l2norm(p); qn = l2norm(q)
    w  = -(pn @ qn^T) / D          # [S,S]
    mv = (w @ q) / S               # [S,D]
    mn = l2norm(mv)
    out = -mean(pn * mn, -1)       # [S]

Folded device pipeline (scalars folded, sign flips cancel):
    qs   = sqrt(1/|q_j|) * q_j                      (host)
    G'   = qs @ p^T                 [S,S]  fp8 DoubleRow matmul (G'[j,i])
    g8   = fp8(G')                  PSUM->SBUF copy
    dot_i = sum_j g8[j,i]^2         square (ACT) + adds + ones-matmul
    M    = sum_j g8[j,i] qs[j,d]    [i,d]  fp8 DoubleRow matmul
    ss_i = sum_d M[i,d]^2           square-accumulate along free dim
    out_i = (1/(D |p_i|)) dot_i / sqrt(ss_i)

Each core handles 8 batches; inputs shipped as fp8(e4m3) in transposed
(d-major) and natural (j-major) layouts; all accumulation fp32.
"""

import os
import sys

for _p in ("/opt/trn_rl_repo",):
    if _p not in sys.path:
        sys.path.append(_p)

import numpy as np
import ml_dtypes

import concourse.bacc as bacc
import concourse.mybir as mybir
import concourse.tile as tile
from concourse.bass_utils import run_bass_kernel_spmd

B, S, D = 64, 512, 768
NCORES = 8
BP = B // NCORES          # batches per core
ST = S // 128             # s tiles (4)
KT = D // 128             # d tiles (6)
DC = 2                    # d chunks for mm2 output (2 x 384)
DW = D // DC              # 384
F32 = mybir.dt.float32
BF16 = mybir.dt.bfloat16
F8 = mybir.dt.float8e4
AF = mybir.ActivationFunctionType
ALU = mybir.AluOpType
DR = mybir.MatmulPerfMode.DoubleRow

_NC = None


def _build(ncores=NCORES, do_compile=True):
    nc = bacc.Bacc("TRN2", target_bir_lowering=False, debug=False, num_devices=ncores)
    # transposed layouts: [b, part, k, s] with d = k*128 + part
    pt_d = nc.dram_tensor("pt8", [BP, 128, KT, S], F8, kind="ExternalInput")
    qt_d = nc.dram_tensor("qt8", [BP, 128, KT, S], F8, kind="ExternalInput")
    # natural layout: [b, part, js, d] with j = js*128 + part
    qh_d = nc.dram_tensor("qh8", [BP, 128, ST, D], F8, kind="ExternalInput")
    # 1/(D*|p_i|) at [part, b*ST + t], i = t*128 + part
    rpc_d = nc.dram_tensor("rpc", [128, BP * ST], F32, kind="ExternalInput")
    out_d = nc.dram_tensor("out", [128, BP * ST], F32, kind="ExternalOutput")

    with tile.TileContext(nc) as tc:
        with (
            tc.tile_pool(name="cst", bufs=1) as cst,
            tc.tile_pool(name="inp", bufs=3) as inp,
            tc.tile_pool(name="g8p", bufs=2) as g8p,
            tc.tile_pool(name="hpp", bufs=2) as hpp,
            tc.tile_pool(name="scr", bufs=2) as scr,
            tc.tile_pool(name="res", bufs=1) as res,
            tc.tile_pool(name="gps", bufs=3, space="PSUM") as gps,
            tc.tile_pool(name="mps", bufs=2, space="PSUM") as mps,
            tc.tile_pool(name="dps", bufs=1, space="PSUM") as dps,
        ):
            ones16 = cst.tile([128, 1], BF16)
            nc.gpsimd.memset(ones16[:], 1.0)
            rpc = cst.tile([128, BP * ST], F32)
            nc.gpsimd.dma_start(rpc[:], rpc_d[:])

            # per-i dot products, transposed: col b*ST + t, i = t*128 + part
            dotT = dps.tile([128, 512], F32)
            # ss accumulator columns [i-part, b*ST + ib]
            ssc = res.tile([128, BP * ST], F32)

            for b in range(BP):
                # batch 0 lands chunked so mm1 starts on the first arrivals
                nch = 3 if b == 0 else 1
                w = KT // nch
                qt_c = []
                pt_c = []
                for c in range(nch):
                    qc = inp.tile([128, w, S], F8, tag=f"qt{c}_{nch}")
                    nc.sync.dma_start(qc[:], qt_d[b, :, c * w:(c + 1) * w, :])
                    pc = inp.tile([128, w, S], F8, tag=f"pt{c}_{nch}")
                    nc.sync.dma_start(pc[:], pt_d[b, :, c * w:(c + 1) * w, :])
                    qt_c.append(qc)
                    pt_c.append(pc)
                qh_t = inp.tile([128, ST, D], F8, tag="qh")
                nc.gpsimd.dma_start(qh_t[:], qh_d[b])

                g8 = g8p.tile([128, ST, S], F8, tag="g8")
                hp = hpp.tile([128, ST, S], BF16, tag="hp")

                # mm1: G'[j,i] = sum_d qs[j,d] p[i,d]  (fp8 DoubleRow, K=256/mm)
                # batch 0: ks-outer over jt pairs to start before all chunks land
                gtiles = {}
                for jtg in range(2):
                    for jt in (2 * jtg, 2 * jtg + 1):
                        gt = gps.tile([128, S], F32, tag="g")
                        gtiles[jt] = gt
                    for ks in range(0, KT, 2):
                        kc, ko = divmod(ks, w) if nch > 1 else (0, ks)
                        for jt in (2 * jtg, 2 * jtg + 1):
                            nc.tensor.matmul(
                                gtiles[jt][:],
                                lhsT=qt_c[kc][:, ko:ko + 2, jt * 128:(jt + 1) * 128],
                                rhs=pt_c[kc][:, ko:ko + 2, :],
                                start=(ks == 0), stop=(ks == KT - 2),
                                perf_mode=DR,
                            )
                    for jt in (2 * jtg, 2 * jtg + 1):
                        g = gtiles[jt]
                        # PSUM -> SBUF fp8 copy (mm2 lhsT); h' ~= G^2 (bf16)
                        nc.vector.tensor_copy(g8[:, jt, :], g[:])
                        if jt == 0:
                            nc.scalar.activation(hp[:, jt, :], g[:], AF.Square)
                        else:
                            nc.vector.scalar_tensor_tensor(
                                hp[:, jt, :], g[:], 1.0, g8[:, jt, :],
                                op0=ALU.mult, op1=ALU.mult,
                            )

                # mm2: M[i,d] = sum_j g8[j,i] qs[j,d]  (fp8 DoubleRow)
                # output i on partitions so ss reduces along free dim
                for ib in range(ST):
                    mc = mps.tile([128, DC, 512], F32, tag="mc")
                    for js in range(0, ST, 2):
                        for dc in range(DC):
                            nc.tensor.matmul(
                                mc[:, dc, 0:DW],
                                lhsT=g8[:, js:js + 2, ib * 128:(ib + 1) * 128],
                                rhs=qh_t[:, js:js + 2, dc * DW:(dc + 1) * DW],
                                start=(js == 0), stop=(js == ST - 2),
                                perf_mode=DR,
                            )
                    col = b * ST + ib
                    # ss: one ACT square + accumulate over both chunks (3D AP)
                    s2 = scr.tile([128, DC * DW], BF16, tag="s2")
                    nc.scalar.activation(
                        s2[:], mc[:, :, 0:DW], AF.Square,
                        accum_out=ssc[:, col:col + 1],
                    )

                # dot: sum h' over j-tiles (GPSIMD adds), then ones-matmuls
                ha = scr.tile([128, 2, S], BF16, tag="ha")
                nc.gpsimd.tensor_tensor(
                    ha[:], hp[:, 0:2, :], hp[:, 2:4, :], ALU.add
                )
                hs = scr.tile([128, S], BF16, tag="hs")
                nc.gpsimd.tensor_tensor(
                    hs[:], ha[:, 0:1, :], ha[:, 1:2, :], ALU.add
                )
                for t in range(ST):
                    col = b * ST + t
                    nc.tensor.matmul(
                        dotT[:, col:col + 1],
                        lhsT=hs[:, t * 128:(t + 1) * 128],
                        rhs=ones16[:],
                        start=(b == 0 and t == 0),
                        stop=(b == BP - 1 and t == ST - 1),
                        skip_group_check=True,
                    )

                # finals in two halves to shorten the tail:
                # out = dot * rpc / sqrt(ss)
                if b in (BP // 2 - 1, BP - 1):
                    h0 = (0 if b < BP // 2 else BP // 2) * ST
                    h1 = (b + 1) * ST
                    sd = res.tile([128, BP * ST], F32, tag="sd")
                    nc.scalar.activation(sd[:, h0:h1], ssc[:, h0:h1], AF.Sqrt)
                    rs = res.tile([128, BP * ST], F32, tag="rs")
                    nc.vector.reciprocal(rs[:, h0:h1], sd[:, h0:h1])
                    w1 = res.tile([128, BP * ST], F32, tag="w1")
                    nc.vector.tensor_tensor(
                        w1[:, h0:h1], dotT[:, h0:h1], rs[:, h0:h1], ALU.mult
                    )
                    wd = res.tile([128, BP * ST], F32, tag="wd")
                    nc.vector.tensor_tensor(
                        wd[:, h0:h1], w1[:, h0:h1], rpc[:, h0:h1], ALU.mult
                    )
                    nc.sync.dma_start(out_d[:, h0:h1], wd[:, h0:h1])
    if do_compile:
        nc.compile()
    return nc


def _get_nc():
    global _NC
    if _NC is None:
        _NC = _build()
    return _NC


F8NP = ml_dtypes.float8_e4m3


def _prep_inputs(p, q):
    p = np.asarray(p, dtype=np.float32)
    q = np.asarray(q, dtype=np.float32)

    nq = np.sqrt((q * q).sum(-1))                 # [B,S]
    srq = (1.0 / np.sqrt(nq))[..., None]          # [B,S,1]
    qs = (q * srq).astype(np.float32)
    npn = np.sqrt((p * p).sum(-1))                # [B,S]
    rpc = (1.0 / (float(D) * npn)).astype(np.float32)

    # transposed: [core, b, part, k, s] with d = k*128 + part
    def tr(x):
        x8 = x.astype(F8NP)
        return np.ascontiguousarray(
            x8.reshape(NCORES, BP, S, KT, 128).transpose(0, 1, 4, 3, 2)
        )

    # natural: [core, b, part, js, d] with j = js*128 + part
    def nat(x):
        x8 = x.astype(F8NP)
        return np.ascontiguousarray(
            x8.reshape(NCORES, BP, ST, 128, D).transpose(0, 1, 3, 2, 4)
        )

    pt8, qt8, qh8 = tr(p), tr(qs), nat(qs)
    # rpc: [core, part, b*ST + t], i = t*128 + part
    rpc_l = np.ascontiguousarray(
        rpc.reshape(NCORES, BP, ST, 128).transpose(0, 3, 1, 2)
    ).reshape(NCORES, 128, BP * ST)
    return [
        {"pt8": pt8[c], "qt8": qt8[c], "qh8": qh8[c], "rpc": rpc_l[c]}
        for c in range(NCORES)
    ]


def _postprocess(results):
    o = np.stack([np.asarray(r["out"], dtype=np.float32) for r in results])
    # o[c, part, b*ST + t] is out for batch c*BP+b at i = t*128 + part
    o = o.reshape(NCORES, 128, BP, ST).transpose(0, 2, 3, 1).reshape(B, 1, S)
    return np.ascontiguousarray(o)


def _run(inputs, trace=False, **kw):
    nc = _get_nc()
    in_maps = _prep_inputs(inputs["p"], inputs["q"])
    res = run_bass_kernel_spmd(nc, in_maps, list(range(NCORES)), trace=trace, **kw)
    return _postprocess(res.results), res


def kernel(p, q):
    out, _ = _run({"p": p, "q": q})
    return out


# revision 23
# speedup vs baseline: 1.9661x; 1.1013x over previous
"""AttentiveMatch kernel for Trainium2 (8 NeuronCores, data-parallel over batch).

Reference math (per batch):
    pn = l2norm(p); qn = l2norm(q)
    w  = -(pn @ qn^T) / D          # [S,S]
    mv = (w @ q) / S               # [S,D]
    mn = l2norm(mv)
    out = -mean(pn * mn, -1)       # [S]

Folded device pipeline (scalars folded, sign flips cancel):
    qs   = sqrt(1/|q_j|) * q_j                      (host)
    G'   = qs @ p^T                 [S,S]  fp8 DoubleRow matmul (G'[j,i])
    g8   = fp8(G')                  PSUM->SBUF copy
    dot_i = sum_j g8[j,i]^2         square (ACT) + adds + ones-matmul
    M    = sum_j g8[j,i] qs[j,d]    [i,d]  fp8 DoubleRow matmul
    ss_i = sum_d M[i,d]^2           square-accumulate along free dim
    out_i = (1/(D |p_i|)) dot_i / sqrt(ss_i)

Each core handles 8 batches; inputs shipped as fp8(e4m3) in transposed
(d-major) and natural (j-major) layouts; all accumulation fp32.
"""

import os
import sys

for _p in ("/opt/trn_rl_repo",):
    if _p not in sys.path:
        sys.path.append(_p)

import numpy as np
import ml_dtypes

import concourse.bacc as bacc
import concourse.mybir as mybir
import concourse.tile as tile
from concourse.bass_utils import run_bass_kernel_spmd

B, S, D = 64, 512, 768
NCORES = 8
BP = B // NCORES          # batches per core
ST = S // 128             # s tiles (4)
KT = D // 128             # d tiles (6)
DC = 2                    # d chunks for mm2 output (2 x 384)
DW = D // DC              # 384
F32 = mybir.dt.float32
BF16 = mybir.dt.bfloat16
F8 = mybir.dt.float8e4
AF = mybir.ActivationFunctionType
ALU = mybir.AluOpType
DR = mybir.MatmulPerfMode.DoubleRow

_NC = None


def _build(ncores=NCORES, do_compile=True):
    nc = bacc.Bacc("TRN2", target_bir_lowering=False, debug=False, num_devices=ncores)
    # transposed layouts: [b, part, k, s] with d = k*128 + part
    pt_d = nc.dram_tensor("pt8", [BP, 128, KT, S], F8, kind="ExternalInput")
    qt_d = nc.dram_tensor("qt8", [BP, 128, KT, S], F8, kind="ExternalInput")
    # natural layout: [b, part, js, d] with j = js*128 + part
    qh_d = nc.dram_tensor("qh8", [BP, 128, ST, D], F8, kind="ExternalInput")
    # 1/(D*|p_i|) at [part, b*ST + t], i = t*128 + part
    rpc_d = nc.dram_tensor("rpc", [128, BP * ST], F32, kind="ExternalInput")
    out_d = nc.dram_tensor("out", [128, BP * ST], F32, kind="ExternalOutput")

    with tile.TileContext(nc) as tc:
        with (
            tc.tile_pool(name="cst", bufs=1) as cst,
            tc.tile_pool(name="inp", bufs=3) as inp,
            tc.tile_pool(name="g8p", bufs=2) as g8p,
            tc.tile_pool(name="hpp", bufs=2) as hpp,
            tc.tile_pool(name="scr", bufs=2) as scr,
            tc.tile_pool(name="res", bufs=1) as res,
            tc.tile_pool(name="gps", bufs=3, space="PSUM") as gps,
            tc.tile_pool(name="mps", bufs=2, space="PSUM") as mps,
            tc.tile_pool(name="dps", bufs=1, space="PSUM") as dps,
        ):
            ones16 = cst.tile([128, 1], BF16)
            nc.gpsimd.memset(ones16[:], 1.0)
            rpc = cst.tile([128, BP * ST], F32)
            nc.gpsimd.dma_start(rpc[:], rpc_d[:])

            # per-i dot products, transposed: col b*ST + t, i = t*128 + part
            dotT = dps.tile([128, 512], F32)
            # ss accumulator columns [i-part, b*ST + ib]
            ssc = res.tile([128, BP * ST], F32)

            for b in range(BP):
                # batch 0 lands chunked so mm1 starts on the first arrivals
                nch = 3 if b == 0 else 1
                w = KT // nch
                qt_c = []
                pt_c = []
                for c in range(nch):
                    qc = inp.tile([128, w, S], F8, tag=f"qt{c}_{nch}")
                    nc.sync.dma_start(qc[:], qt_d[b, :, c * w:(c + 1) * w, :])
                    pc = inp.tile([128, w, S], F8, tag=f"pt{c}_{nch}")
                    if b == 0:
                        nc.scalar.dma_start(pc[:], pt_d[b, :, c * w:(c + 1) * w, :])
                    else:
                        nc.sync.dma_start(pc[:], pt_d[b, :, c * w:(c + 1) * w, :])
                    qt_c.append(qc)
                    pt_c.append(pc)
                qh_t = inp.tile([128, ST, D], F8, tag="qh")
                nc.gpsimd.dma_start(qh_t[:], qh_d[b])

                g8 = g8p.tile([128, ST, S], F8, tag="g8")
                hp = hpp.tile([128, ST, S], BF16, tag="hp")

                # mm1: G'[j,i] = sum_d qs[j,d] p[i,d]  (fp8 DoubleRow, K=256/mm)
                # batch 0: ks-outer over jt pairs to start before all chunks
                # land; later batches jt-outer so g8 casts trail each jt tile
                def mm1_drain(jt, g):
                    # PSUM -> SBUF fp8 copy (mm2 lhsT); h' ~= G^2 (bf16)
                    nc.vector.tensor_copy(g8[:, jt, :], g[:])
                    if jt == 0:
                        nc.scalar.activation(hp[:, jt, :], g[:], AF.Square)
                    else:
                        nc.vector.scalar_tensor_tensor(
                            hp[:, jt, :], g[:], 1.0, g8[:, jt, :],
                            op0=ALU.mult, op1=ALU.mult,
                        )
                    if jt == 1:
                        # partial pair-sum early (slow engine, hidden)
                        nc.gpsimd.tensor_tensor(
                            ha[:, 0:1, :], hp[:, 0:1, :], hp[:, 1:2, :], ALU.add
                        )

                ha = scr.tile([128, 2, S], BF16, tag="ha")
                if b == 0:
                    gtiles = {}
                    for jtg in range(2):
                        for jt in (2 * jtg, 2 * jtg + 1):
                            gt = gps.tile([128, S], F32, tag="g")
                            gtiles[jt] = gt
                        for ks in range(0, KT, 2):
                            kc, ko = divmod(ks, w)
                            for jt in (2 * jtg, 2 * jtg + 1):
                                nc.tensor.matmul(
                                    gtiles[jt][:],
                                    lhsT=qt_c[kc][:, ko:ko + 2, jt * 128:(jt + 1) * 128],
                                    rhs=pt_c[kc][:, ko:ko + 2, :],
                                    start=(ks == 0), stop=(ks == KT - 2),
                                    perf_mode=DR,
                                )
                        for jt in (2 * jtg, 2 * jtg + 1):
                            mm1_drain(jt, gtiles[jt])
                else:
                    for jt in range(ST):
                        g = gps.tile([128, S], F32, tag="g")
                        for ks in range(0, KT, 2):
                            nc.tensor.matmul(
                                g[:],
                                lhsT=qt_c[0][:, ks:ks + 2, jt * 128:(jt + 1) * 128],
                                rhs=pt_c[0][:, ks:ks + 2, :],
                                start=(ks == 0), stop=(ks == KT - 2),
                                perf_mode=DR,
                            )
                        mm1_drain(jt, g)

                # mm2: M[i,d] = sum_j g8[j,i] qs[j,d]  (fp8 DoubleRow)
                # output i on partitions so ss reduces along free dim
                for ib in range(ST):
                    mc = mps.tile([128, DC, 512], F32, tag="mc")
                    for js in range(0, ST, 2):
                        for dc in range(DC):
                            nc.tensor.matmul(
                                mc[:, dc, 0:DW],
                                lhsT=g8[:, js:js + 2, ib * 128:(ib + 1) * 128],
                                rhs=qh_t[:, js:js + 2, dc * DW:(dc + 1) * DW],
                                start=(js == 0), stop=(js == ST - 2),
                                perf_mode=DR,
                            )
                    col = b * ST + ib
                    # ss: one ACT square + accumulate over both chunks (3D AP)
                    s2 = scr.tile([128, DC * DW], BF16, tag="s2")
                    nc.scalar.activation(
                        s2[:], mc[:, :, 0:DW], AF.Square,
                        accum_out=ssc[:, col:col + 1],
                    )

                # dot: finish h' pair-sums (DVE bf16 2x, short critical path)
                nc.vector.tensor_tensor(
                    ha[:, 1:2, :], hp[:, 2:3, :], hp[:, 3:4, :], ALU.add
                )
                hs = scr.tile([128, S], BF16, tag="hs")
                nc.vector.tensor_tensor(
                    hs[:], ha[:, 0:1, :], ha[:, 1:2, :], ALU.add
                )
                for t in range(ST):
                    col = b * ST + t
                    nc.tensor.matmul(
                        dotT[:, col:col + 1],
                        lhsT=hs[:, t * 128:(t + 1) * 128],
                        rhs=ones16[:],
                        start=(b == 0 and t == 0),
                        stop=(b == BP - 1 and t == ST - 1),
                        skip_group_check=True,
                    )

                # finals in two halves to shorten the tail:
                # out = dot * rpc / sqrt(ss)
                if b in (BP // 2 - 1, BP - 1):
                    h0 = (0 if b < BP // 2 else BP // 2) * ST
                    h1 = (b + 1) * ST
                    sd = res.tile([128, BP * ST], F32, tag="sd")
                    nc.scalar.activation(sd[:, h0:h1], ssc[:, h0:h1], AF.Sqrt)
                    rs = res.tile([128, BP * ST], F32, tag="rs")
                    nc.vector.reciprocal(rs[:, h0:h1], sd[:, h0:h1])
                    w1 = res.tile([128, BP * ST], F32, tag="w1")
                    nc.vector.tensor_tensor(
                        w1[:, h0:h1], dotT[:, h0:h1], rs[:, h0:h1], ALU.mult
                    )
                    wd = res.tile([128, BP * ST], F32, tag="wd")
                    nc.vector.tensor_tensor(
                        wd[:, h0:h1], w1[:, h0:h1], rpc[:, h0:h1], ALU.mult
                    )
                    nc.sync.dma_start(out_d[:, h0:h1], wd[:, h0:h1])
    if do_compile:
        nc.compile()
    return nc


def _get_nc():
    global _NC
    if _NC is None:
        _NC = _build()
    return _NC


F8NP = ml_dtypes.float8_e4m3


def _prep_inputs(p, q):
    p = np.asarray(p, dtype=np.float32)
    q = np.asarray(q, dtype=np.float32)

    nq = np.sqrt((q * q).sum(-1))                 # [B,S]
    srq = (1.0 / np.sqrt(nq))[..., None]          # [B,S,1]
    qs = (q * srq).astype(np.float32)
    npn = np.sqrt((p * p).sum(-1))                # [B,S]
    rpc = (1.0 / (float(D) * npn)).astype(np.float32)

    # transposed: [core, b, part, k, s] with d = k*128 + part
    def tr(x):
        x8 = x.astype(F8NP)
        return np.ascontiguousarray(
            x8.reshape(NCORES, BP, S, KT, 128).transpose(0, 1, 4, 3, 2)
        )

    # natural: [core, b, part, js, d] with j = js*128 + part
    def nat(x):
        x8 = x.astype(F8NP)
        return np.ascontiguousarray(
            x8.reshape(NCORES, BP, ST, 128, D).transpose(0, 1, 3, 2, 4)
        )

    pt8, qt8, qh8 = tr(p), tr(qs), nat(qs)
    # rpc: [core, part, b*ST + t], i = t*128 + part
    rpc_l = np.ascontiguousarray(
        rpc.reshape(NCORES, BP, ST, 128).transpose(0, 3, 1, 2)
    ).reshape(NCORES, 128, BP * ST)
    return [
        {"pt8": pt8[c], "qt8": qt8[c], "qh8": qh8[c], "rpc": rpc_l[c]}
        for c in range(NCORES)
    ]


def _postprocess(results):
    o = np.stack([np.asarray(r["out"], dtype=np.float32) for r in results])
    # o[c, part, b*ST + t] is out for batch c*BP+b at i = t*128 + part
    o = o.reshape(NCORES, 128, BP, ST).transpose(0, 2, 3, 1).reshape(B, 1, S)
    return np.ascontiguousarray(o)


def _run(inputs, trace=False, **kw):
    nc = _get_nc()
    in_maps = _prep_inputs(inputs["p"], inputs["q"])
    res = run_bass_kernel_spmd(nc, in_maps, list(range(NCORES)), trace=trace, **kw)
    return _postprocess(res.results), res


def kernel(p, q):
    out, _ = _run({"p": p, "q": q})
    return out
